# revision 1
# baseline (speedup 1.0000x reference)
"""Trainium2 Bass kernel for nn_DeformConv2d_for_Style.

Data-parallel over batch B=8 across 8 NeuronCores (one image per core).
Per core:
  1. Offset conv (reflect-pad) as 9 accumulating fp16 matmuls per 512-sample
     tile into [18, 512] PSUM, then PE transposes into TWO sample-major
     layouts: offT (lane-major, feeds the index chain) and offTQ
     (twist-major, feeds the weight chain aligned with gather partitions).
  2. Index chain P1 on DVE ([128, 288] fp32, cols = kk*32 + t): sampling
     positions, clamped floor, int16 gather row indices (x-major table).
     Weight chain P2 ([128, 288], cols = h*36 + kk*4 + thi): bilinear
     weights (no masks: the zero-padded table makes OOB taps exact zeros).
  3. Gathers chunked BY LANE GROUP h (j = h*288 + kk*32 + t), so the int16
     index wrap is 8 clean contiguous block-DMAs instead of thousands of
     2-byte strided descriptors. dma_gather fp16 elem=512 (2x2-patch rows:
     4 bilinear taps x 128 ch per 1KB descriptor), 2 sub-gathers per chunk
     on rotating SWDGE queues.
  4. Bilinear reduce via scalar_tensor_tensor chains (per-partition scalar
     weights in the twist layout), PE transpose -> cols [channel, item].
  5. Main conv: fp16 matmuls with 3D-strided moving APs that restore raster
     sample order in PSUM; biased fp16 output accumulated in SBUF, two
     final contiguous output DMAs.

All static layout prep (padding, x-major 2x2-patch table, fp16 casts,
index base tables for both layouts, identities) happens host-side.
"""

import dataclasses
import sys

import numpy as np

sys.path.insert(0, "/opt/trn_rl_repo")

B, CIN, COUT, K = 8, 128, 256, 3
KK = K * K
H = W = 64
HW = H * W
NT = HW // 128          # 32 l-tiles of 128 samples
G = NT * KK             # 288 = columns of the stage-2 tiles
PW = W + 2              # 66
TROWS = 4608            # zero-padded x-major table rows (66*68 area, 4488 used)
NH = 8                  # chunks = lane groups
GC = G // NH * 1        # 36 blocks per chunk  (= 288 j-slots / 8)

_CACHE = {}


def _build_program(loop_n=None, variant=None):
    import concourse.tile as tile
    from concourse import bacc, mybir

    f16 = mybir.dt.float16
    f32 = mybir.dt.float32
    i16 = mybir.dt.int16
    i32 = mybir.dt.int32
    Alu = mybir.AluOpType
    Act = mybir.ActivationFunctionType

    nc = bacc.Bacc("TRN2", target_bir_lowering=False, debug=False,
                   num_swdge_queues=4)

    # ---- DRAM parameters (per-core) ----
    xrow_d = nc.dram_tensor("xrow", [CIN, 3 * PW * W], f16, kind="ExternalInput")
    xt_d = nc.dram_tensor("xt", [TROWS * 4 * CIN], f16, kind="ExternalInput")
    wofft_d = nc.dram_tensor("wofft", [CIN, KK * 2 * KK], f16, kind="ExternalInput")
    wdct_d = nc.dram_tensor("wdct", [CIN, KK * COUT], f16, kind="ExternalInput")
    bdc_d = nc.dram_tensor("bdc", [128, 2], f32, kind="ExternalInput")
    pyb1_d = nc.dram_tensor("pyb1", [128, G], f32, kind="ExternalInput")
    pxb1_d = nc.dram_tensor("pxb1", [128, G], f32, kind="ExternalInput")
    pyb2_d = nc.dram_tensor("pyb2", [128, G], f32, kind="ExternalInput")
    pxb2_d = nc.dram_tensor("pxb2", [128, G], f32, kind="ExternalInput")
    id16_d = nc.dram_tensor("id16", [128, 128], f16, kind="ExternalInput")
    id32_d = nc.dram_tensor("id32", [32, 32], f32, kind="ExternalInput")
    out_d = nc.dram_tensor("out", [COUT, HW], f16, kind="ExternalOutput")

    # gather source view: x-major 2x2-patch rows, 4*128 ch each
    gsrc = dataclasses.replace(xt_d.ap(), ap=[[4 * CIN, TROWS - 1], [1, 4 * CIN]])

    from contextlib import ExitStack, nullcontext

    with tile.TileContext(nc) as tc, ExitStack() as ctx:
        cpool = ctx.enter_context(tc.tile_pool(name="consts", bufs=1))
        wpool = ctx.enter_context(tc.tile_pool(name="work", bufs=1))
        gpool = ctx.enter_context(tc.tile_pool(name="gath", bufs=2))
        zpool = ctx.enter_context(tc.tile_pool(name="z", bufs=4))
        colpool = ctx.enter_context(tc.tile_pool(name="cols", bufs=2))
        opool = ctx.enter_context(tc.tile_pool(name="outs", bufs=1))
        ps_off = ctx.enter_context(tc.tile_pool(name="ps_off", bufs=2, space="PSUM"))
        ps_tr = ctx.enter_context(tc.tile_pool(name="ps_tr", bufs=2, space="PSUM"))
        ps_mm = ctx.enter_context(tc.tile_pool(name="ps_mm", bufs=2, space="PSUM"))

        loop_cm = tc.For_i(0, loop_n, 1) if loop_n else nullcontext()
        with loop_cm:
            if variant == "empty":
                em = cpool.tile([128, 128], f16)
                nc.vector.memset(em[:], 0.0)
                nc.sync.dma_start(out_d.ap()[0:128, 0:128], em[:])
            else:
                # ---- constants ----
                xrow = cpool.tile([CIN, 3 * PW * W], f16)
                nc.sync.dma_start(xrow[:], xrow_d.ap())
                wofft = cpool.tile([CIN, KK * 2 * KK], f16)
                nc.sync.dma_start(wofft[:], wofft_d.ap())
                wdct = cpool.tile([CIN, KK * COUT], f16)
                nc.sync.dma_start(wdct[:], wdct_d.ap())
                bdc = cpool.tile([128, 2], f32)
                nc.sync.dma_start(bdc[:], bdc_d.ap())
                pyb1 = cpool.tile([128, G], f32)
                nc.sync.dma_start(pyb1[:], pyb1_d.ap())
                pxb1 = cpool.tile([128, G], f32)
                nc.sync.dma_start(pxb1[:], pxb1_d.ap())
                pyb2 = cpool.tile([128, G], f32)
                nc.sync.dma_start(pyb2[:], pyb2_d.ap())
                pxb2 = cpool.tile([128, G], f32)
                nc.sync.dma_start(pxb2[:], pxb2_d.ap())
                id16 = cpool.tile([128, 128], f16)
                nc.sync.dma_start(id16[:], id16_d.ap())
                id32 = cpool.tile([32, 32], f32)
                nc.sync.dma_start(id32[:], id32_d.ap())

                # ---- stage 1: offset conv -> off_sb [18, HW] ----
                off_sb = wpool.tile([18, HW], f32, name="off_sb")
                for nt in range(8):
                    po = ps_off.tile([18, 512], f32, name="po", tag="po")
                    for kk in range(KK):
                        ki, kj = kk // 3, kk % 3
                        base = kj * PW * W + (nt * 8 + ki) * W
                        nc.tensor.matmul(
                            po[:], wofft[:, kk * 18:(kk + 1) * 18],
                            xrow[:, base:base + 512],
                            start=(kk == 0), stop=(kk == KK - 1),
                        )
                    nc.scalar.copy(off_sb[:, nt * 512:(nt + 1) * 512], po[:])

                # lane-major transposes: offT[L, t*18 + coef]
                offT = wpool.tile([128, NT * 2 * KK], f32)
                for t in range(NT):
                    pT = ps_off.tile([128, 2 * KK], f32, name="pT", tag="pT")
                    nc.tensor.transpose(
                        pT[:], off_sb[:, t * 128:(t + 1) * 128], id32[0:18, 0:18])
                    nc.scalar.copy(offT[:, t * 18:(t + 1) * 18], pT[:])

                # twist-major transposes: offTQ[(tlo,p'), (h*4+thi)*18 + coef]
                offTQ = wpool.tile([128, NT * 2 * KK], f32)
                osv = off_sb[:].rearrange("c (t l) -> c t l", l=128)
                for h8 in range(NH):
                    for thi in range(4):
                        blk = osv[:, thi * 8:(thi + 1) * 8,
                                  h8 * 16:(h8 + 1) * 16]
                        qtmp = zpool.tile([18, 128], f32, tag="qblk")
                        nc.vector.tensor_copy(
                            qtmp[:].rearrange("c (a b) -> c a b", a=8), blk)
                        pTQ = ps_off.tile([128, 2 * KK], f32, name="pTQ",
                                          tag="pT")
                        nc.tensor.transpose(pTQ[:], qtmp[:], id32[0:18, 0:18])
                        q = h8 * 4 + thi
                        nc.scalar.copy(offTQ[:, q * 18:(q + 1) * 18], pTQ[:])

                def wtile(tag, dt=f32):
                    return wpool.tile([128, G], dt, tag=tag, name=tag)

                def clamp_pypx(p):
                    nc.vector.tensor_scalar(p[:], p[:], -1.0, None, Alu.max)
                    nc.vector.tensor_scalar(p[:], p[:], 64.75, None, Alu.min)

                def floor_of(p, tag):
                    ti = wpool.tile([128, G], i32, tag="fl_i", name="fl_i")
                    nc.vector.tensor_copy(ti[:], p[:])
                    tf = wtile(tag)
                    nc.vector.tensor_copy(tf[:], ti[:])
                    gt = wpool.tile([128, G], f32, tag="fl_g", name="fl_g")
                    nc.vector.tensor_tensor(gt[:], tf[:], p[:], Alu.is_gt)
                    nc.vector.tensor_tensor(tf[:], tf[:], gt[:], Alu.subtract)
                    return tf

                # ---- P1: index chain (cols g1 = kk*32 + t) ----
                o1 = offT[:].rearrange("p (t k u) -> p k t u", k=KK, u=2)
                dy1 = o1[:, :, :, 0]
                dx1 = o1[:, :, :, 1]

                def v1(ap):
                    return ap.rearrange("p (k t) -> p k t", k=KK)

                py1 = wtile("py1")
                nc.vector.tensor_tensor(v1(py1[:]), dy1, v1(pyb1[:]), Alu.add)
                px1 = wtile("px1")
                nc.vector.tensor_tensor(v1(px1[:]), dx1, v1(pxb1[:]), Alu.add)
                clamp_pypx(py1)
                clamp_pypx(px1)
                y01 = floor_of(py1, "y01")
                x01 = floor_of(px1, "x01")
                idxf = wtile("idxf")
                nc.vector.scalar_tensor_tensor(
                    idxf[:], x01[:], 66.0, y01[:], Alu.mult, Alu.add)
                nc.vector.tensor_scalar(idxf[:], idxf[:], 67.0, None, Alu.add)
                idx_i32 = wpool.tile([128, G], i32)
                nc.vector.tensor_copy(idx_i32[:], idxf[:])
                idx16 = wpool.tile([128, G], i16)
                nc.vector.tensor_copy(idx16[:], idx_i32[:])

                # ---- wrap: per-h block copy + per-h replication, alternating
                # HWDGE rings, so chunk h's gather only waits on its own
                # 4 small DMAs instead of the whole wrap ----
                wrap_t = wpool.tile([128, G * 8], i16)
                for h8 in range(NH):
                    eng_d = nc.sync if h8 % 2 == 0 else nc.scalar
                    cols = slice(h8 * G, (h8 + 1) * G)
                    eng_d.dma_start(wrap_t[0:16, cols],
                                    idx16[h8 * 16:(h8 + 1) * 16, :])
                    for r in (16, 32, 64):
                        eng_d.dma_start(wrap_t[r:2 * r, cols],
                                        wrap_t[0:r, cols])

                # ---- P2: weight chain (cols = h*36 + kk*4 + thi) ----
                o2 = offTQ[:].rearrange("p (h ti k u) -> p h k ti u",
                                        ti=4, k=KK, u=2)
                dy2 = o2[:, :, :, :, 0]
                dx2 = o2[:, :, :, :, 1]

                def v2(ap):
                    return ap.rearrange("p (h k ti) -> p h k ti", k=KK, ti=4)

                py2 = wtile("py2")
                nc.vector.tensor_tensor(v2(py2[:]), dy2, v2(pyb2[:]), Alu.add)
                px2 = wtile("px2")
                nc.vector.tensor_tensor(v2(px2[:]), dx2, v2(pxb2[:]), Alu.add)
                clamp_pypx(py2)
                clamp_pypx(px2)
                y02 = floor_of(py2, "y02")
                x02 = floor_of(px2, "x02")
                fy = wtile("fy")
                nc.vector.tensor_tensor(fy[:], py2[:], y02[:], Alu.subtract)
                fx = wtile("fx")
                nc.vector.tensor_tensor(fx[:], px2[:], x02[:], Alu.subtract)
                ufy = wtile("ufy")
                nc.vector.tensor_scalar(ufy[:], fy[:], -1.0, 1.0, Alu.mult, Alu.add)
                ufx = wtile("ufx")
                nc.vector.tensor_scalar(ufx[:], fx[:], -1.0, 1.0, Alu.mult, Alu.add)
                w00 = wtile("w00")
                nc.vector.tensor_tensor(w00[:], ufy[:], ufx[:], Alu.mult)
                w01 = wtile("w01")
                nc.vector.tensor_tensor(w01[:], ufy[:], fx[:], Alu.mult)
                w10 = wtile("w10")
                nc.vector.tensor_tensor(w10[:], fy[:], ufx[:], Alu.mult)
                w11 = wtile("w11")
                nc.vector.tensor_tensor(w11[:], fy[:], fx[:], Alu.mult)

                # ---- stage 3: per lane-group chunk ----
                osb = opool.tile([128, 2 * HW], f16, name="osb")
                ov = osb[:].rearrange("p (o ti tl hh s) -> p o ti tl hh s",
                                      o=2, ti=4, tl=8, s=16)
                for h8 in range(NH):
                    gth = gpool.tile([128, GC, 4 * CIN], f16, tag="gth")
                    if variant == "nogather":
                        nc.gpsimd.memset(gth[:], 0.25)
                    else:
                        hgv = gth[:].rearrange("p (u g) e -> p u g e", u=4)
                        for u in range(4):
                            nc.gpsimd.dma_gather(
                                hgv[:, u], gsrc,
                                wrap_t[:, h8 * G + u * (G // 4):
                                       h8 * G + (u + 1) * (G // 4)],
                                GC * 32, GC * 32, 4 * CIN, elem_step=4 * CIN,
                                single_packet=False,
                                queue_num=(4 * h8 + u) % 4,
                            )

                    colsh = colpool.tile([128, GC * 128], f16)
                    pt = None
                    for Bb in range(GC):
                        col = h8 * GC + Bb
                        if variant == "noz":
                            z4 = zpool.tile([128, 128], f16, tag="z4")
                            nc.vector.tensor_copy(z4[:], gth[:, Bb, 0:CIN])
                            if Bb % 4 == 0:
                                pt = ps_tr.tile([128, 512], f16)
                            nc.tensor.transpose(
                                pt[:, (Bb % 4) * 128:(Bb % 4 + 1) * 128],
                                z4[:], id16[:])
                            if Bb % 4 == 3:
                                nc.vector.tensor_copy(
                                    colsh[:, (Bb - 3) * 128:(Bb + 1) * 128],
                                    pt[:])
                            continue
                        z1 = zpool.tile([128, 128], f16, tag="z1")
                        nc.scalar.mul(z1[:], gth[:, Bb, 0:CIN],
                                      w00[:, col:col + 1])
                        z2 = zpool.tile([128, 128], f16, tag="z2")
                        if Bb % 2 == 0:
                            # ACT-heavy form: 2nd mul on ACT, combine with a
                            # 2x-mode DVE add (balances DVE vs ACT load)
                            z2a = zpool.tile([128, 128], f16, tag="z2a")
                            nc.scalar.mul(z2a[:], gth[:, Bb, CIN:2 * CIN],
                                          w01[:, col:col + 1])
                            nc.vector.tensor_tensor(
                                z2[:], z1[:], z2a[:], Alu.add)
                        else:
                            nc.vector.scalar_tensor_tensor(
                                z2[:], gth[:, Bb, CIN:2 * CIN],
                                w01[:, col:col + 1], z1[:], Alu.mult, Alu.add)
                        z3 = zpool.tile([128, 128], f16, tag="z3")
                        nc.vector.scalar_tensor_tensor(
                            z3[:], gth[:, Bb, 2 * CIN:3 * CIN],
                            w10[:, col:col + 1], z2[:], Alu.mult, Alu.add)
                        z4 = zpool.tile([128, 128], f16, tag="z4")
                        nc.vector.scalar_tensor_tensor(
                            z4[:], gth[:, Bb, 3 * CIN:4 * CIN],
                            w11[:, col:col + 1], z3[:], Alu.mult, Alu.add)
                        if Bb % 4 == 0:
                            pt = ps_tr.tile([128, 512], f16)
                        nc.tensor.transpose(
                            pt[:, (Bb % 4) * 128:(Bb % 4 + 1) * 128],
                            z4[:], id16[:])
                        if Bb % 4 == 3:
                            nc.vector.tensor_copy(
                                colsh[:, (Bb - 3) * 128:(Bb + 1) * 128], pt[:])

                    # cols for kk are contiguous: col = (kk*4+thi)*128+tlo*16+p'
                    for ot in range(2):
                        pm = ps_mm.tile([128, 512], f32)
                        for kk in range(KK):
                            nc.tensor.matmul(
                                pm[:],
                                wdct[:, kk * COUT + ot * 128:
                                     kk * COUT + (ot + 1) * 128],
                                colsh[:, kk * 512:(kk + 1) * 512],
                                start=(kk == 0), stop=(kk == KK - 1),
                            )
                        pmv = pm[:].rearrange("p (ti tl s) -> p ti tl s",
                                              ti=4, tl=8, s=16)
                        nc.scalar.activation(
                            ov[:, ot, :, :, h8, :], pmv, Act.Identity,
                            bias=bdc[:, ot:ot + 1], scale=1.0)

                for ot in range(2):
                    nc.sync.dma_start(
                        out_d.ap()[ot * 128:(ot + 1) * 128, :],
                        osb[:, ot * HW:(ot + 1) * HW])

    nc.compile()
    return nc


def _host_prep(x, w_off, b_off, w_dc, b_dc):
    """Build per-core input maps (all static layout prep in numpy)."""
    f16 = np.float16
    KI = np.arange(KK) // 3
    KJ = np.arange(KK) % 3

    wofft = np.zeros((CIN, KK * 2 * KK), f16)
    for kk in range(KK):
        wofft[:, kk * 18:(kk + 1) * 18] = w_off[:, :, KI[kk], KJ[kk]].T.astype(f16)
    wdct = np.zeros((CIN, KK * COUT), f16)
    for kk in range(KK):
        wdct[:, kk * COUT:(kk + 1) * COUT] = w_dc[:, :, KI[kk], KJ[kk]].T.astype(f16)
    bdc = np.ascontiguousarray(b_dc.reshape(2, 128).T).astype(np.float32)

    lane = np.arange(128)
    t = np.arange(NT)
    i_of = (t[:, None] * 128 + lane[None, :]) // 64      # [t, lane]
    j_of = (t[:, None] * 128 + lane[None, :]) % 64

    # P1 bases: [L, kk*32 + t]
    pyb1 = np.zeros((128, G), np.float32)
    pxb1 = np.zeros((128, G), np.float32)
    for kk in range(KK):
        pyb1[:, kk * NT:(kk + 1) * NT] = (i_of + KI[kk] - 1 + b_off[2 * kk]).T
        pxb1[:, kk * NT:(kk + 1) * NT] = (j_of + KJ[kk] - 1 + b_off[2 * kk + 1]).T

    # P2 bases: [Q = tlo*16+p', h*36 + kk*4 + thi]
    pyb2 = np.zeros((128, G), np.float32)
    pxb2 = np.zeros((128, G), np.float32)
    tlo = np.arange(128) // 16          # per Q
    pp = np.arange(128) % 16
    for h8 in range(NH):
        for kk in range(KK):
            for thi in range(4):
                col = h8 * 36 + kk * 4 + thi
                tt = thi * 8 + tlo                  # [Q]
                L = h8 * 16 + pp                    # [Q]
                s = tt * 128 + L
                pyb2[:, col] = s // 64 + KI[kk] - 1 + b_off[2 * kk]
                pxb2[:, col] = s % 64 + KJ[kk] - 1 + b_off[2 * kk + 1]

    id16 = np.eye(128, dtype=f16)
    id32 = np.eye(32, dtype=np.float32)

    common = {
        "wofft": wofft, "wdct": wdct, "bdc": bdc,
        "pyb1": pyb1, "pxb1": pxb1, "pyb2": pyb2, "pxb2": pxb2,
        "id16": id16, "id32": id32,
    }
    in_maps = []
    for b in range(B):
        xb = x[b]
        xpad = np.pad(xb, ((0, 0), (1, 1), (1, 1)), mode="reflect").astype(f16)
        xrow = np.stack([xpad[:, :, kj:kj + W] for kj in range(3)], axis=1)
        # x-major 2x2-patch table: row r = x2*66 + y2 holds [v(y,x), v(y,x+1),
        # v(y+1,x), v(y+1,x+1)] each 128 ch; from the zero-padded image
        xz = np.zeros((68, 68, CIN), f16)
        xz[1:65, 1:65, :] = xb.transpose(1, 2, 0).astype(f16)
        t00 = xz[0:66, 0:66]
        t01 = xz[0:66, 1:67]
        t10 = xz[1:67, 0:66]
        t11 = xz[1:67, 1:67]
        tab = np.concatenate([t00, t01, t10, t11], axis=2)
        tab = np.ascontiguousarray(tab.transpose(1, 0, 2)).reshape(-1, 4 * CIN)
        tab = np.concatenate(
            [tab, np.zeros((TROWS - 66 * 66, 4 * CIN), f16)], 0)
        in_maps.append({
            "xrow": np.ascontiguousarray(xrow.reshape(CIN, 3 * PW * W)),
            "xt": np.ascontiguousarray(tab.reshape(-1)),
            **common,
        })
    return in_maps


def kernel(x, w_off, b_off, w_dc, b_dc):
    x = np.asarray(x, dtype=np.float32)
    w_off = np.asarray(w_off, dtype=np.float32)
    b_off = np.asarray(b_off, dtype=np.float32)
    w_dc = np.asarray(w_dc, dtype=np.float32)
    b_dc = np.asarray(b_dc, dtype=np.float32)

    if "nc" not in _CACHE:
        _CACHE["nc"] = _build_program()
    nc = _CACHE["nc"]

    from concourse.bass_utils import run_bass_kernel_spmd

    in_maps = _host_prep(x, w_off, b_off, w_dc, b_dc)
    res = run_bass_kernel_spmd(nc, in_maps, list(range(B)))
    out = np.stack([res.results[b]["out"].reshape(COUT, H, W) for b in range(B)])
    return out.astype(np.float32)


if __name__ == "__main__":
    rng = np.random.default_rng(0)
    x = rng.standard_normal((B, CIN, H, W), dtype=np.float32)
    w_off = rng.standard_normal((2 * KK, CIN, K, K), dtype=np.float32) / 34
    b_off = rng.standard_normal((2 * KK,), dtype=np.float32) * 0.01
    w_dc = rng.standard_normal((COUT, CIN, K, K), dtype=np.float32) / 34
    b_dc = rng.standard_normal((COUT,), dtype=np.float32) * 0.01
    out = kernel(x, w_off, b_off, w_dc, b_dc)
    print("out", out.shape, out.dtype, float(np.abs(out).mean()))

